# revision 45
# baseline (speedup 1.0000x reference)
"""Trainium2 Bass kernel for nn_SubspaceLinopFactory (subspace NUDFT forward).

Math (reference):
  s[a,c,h,w] = x[a,h,w] * mps[c,h,w]
  E[r,k,(h,w)] = exp(-i*(ty[k]*gy[h] + tx[k]*gx[w]))   (separable)
  y[a,c,k] = sum_hw E * s          (per core r)
  z[t,c,k] = sum_a phi[a,t] * y[a,c,k] * dcf[k]
  out[t,c,k] = z from core subsamp_idx[t]

Sharding: trajectory r -> core r (R == 8 == n_cores).

Device pipeline per core (all trig tables precomputed on host, fp16):
  stage-1 (TensorE): [P|Q][(ac,h), k] = sT[w,ach]^T @ (dcf*cos_x | dcf*sin_x)
    -> one [128,1024] PSUM tile (2 banks) per m-tile j (6 m-tiles, KC=512).
  cast (ScalarE): [P|Q] PSUM -> SBUF fp16, one dual op per j.
  products (DVE/Pool): prA = pc * (cy, -sy) = [A|D''], prB = qc * (-sy, -cy)
    = [B''|C''] -- dual fp16 ops with a stride-0 broadcast of pc/qc.
  h-reduction + phi fused (TensorE): weights PH[p, t*4+c'] = phi[a(p),t] for
    c'==c(p); 4 accumulating matmuls per j into z = [z_re|z_im] [128,1024]
    PSUM; y_re = sum(A-B), y_im = -sum(C+D) realized via the sign-packed
    tables, so all streams use +PH.
  z copy PSUM -> SBUF fp16 (z_re on Scalar, z_im on DVE), DMA out per
  k-chunk. PE warm-up matmuls bridge the DMA lead-in to preserve the
  DVFS ramp. Host gathers out[t] from core subsamp_idx[t] rows t*4+c.
"""
import numpy as np

A, T, C, R, D, K, H, W = 3, 32, 4, 8, 2, 1024, 64, 64
N_CORES = 8
AC = A * C               # 12
ACH = AC * H             # 768
MT = ACH // 128          # 6 m-tiles
KC = 512                 # k-chunk (one PSUM bank of f32)
NKC = K // KC            # 2
N_WARM_BIG = 20          # 256-row warm-up matmuls (213ns each at mid clock)
N_WARM_SMALL = 32        # 64-row tail warm-ups (fine-grained bridge to data)
N_FILL = 10              # 64-row gap fillers before the slot-2 pq-buffer wait

_CACHE = {}


def _build_nc():
    import concourse.bacc as bacc
    import concourse.tile as tile
    import concourse.mybir as mybir

    F32 = mybir.dt.float32
    F16 = mybir.dt.float16
    OP = mybir.AluOpType

    nc = bacc.Bacc(None, target_bir_lowering=False)

    # parallel DMA queues; combined lead DMAs (per-DMA fixed cost ~0.6-1us
    # on the queue outweighs smaller transfers — don't over-split)
    d_st = nc.dram_tensor("st", [64, ACH], F16, kind="ExternalInput")
    d_xt0 = nc.dram_tensor("xt0", [64, 2, KC], F16, kind="ExternalInput")
    d_xt1 = nc.dram_tensor("xt1", [64, 2, KC], F16, kind="ExternalInput")
    d_yt0a = nc.dram_tensor("yt0a", [128, 2, KC], F16, kind="ExternalInput")
    d_yt0b = nc.dram_tensor("yt0b", [128, 2, KC], F16, kind="ExternalInput")
    d_ph = nc.dram_tensor("ph", [128, MT * 128], F16, kind="ExternalInput")
    d_yt1 = nc.dram_tensor("yt1", [128, 2, 2, KC], F16, kind="ExternalInput")
    d_z = nc.dram_tensor("z", [128, 2, K], F16, kind="ExternalOutput")

    with tile.TileContext(nc) as tc:
        with (
            tc.tile_pool(name="cst", bufs=1) as cst,
            tc.tile_pool(name="ctp", bufs=3) as ctp,
            tc.tile_pool(name="prp", bufs=5) as prp,
            tc.tile_pool(name="zsb", bufs=2) as zsb,
            tc.tile_pool(name="pq", bufs=2, space="PSUM") as pqp,
            tc.tile_pool(name="zps", bufs=2, space="PSUM") as zpp,
        ):
            dm = cst.tile([64, 256], F16)
            dm2 = cst.tile([64, 2], F16)
            nc.vector.memset(dm[:], 0.0)

            st = cst.tile([64, ACH], F16)
            xt0 = cst.tile([64, 2, KC], F16)
            xt1 = cst.tile([64, 2, KC], F16)
            yt0 = cst.tile([128, 2, 2, KC], F16)
            yt1 = cst.tile([128, 2, 2, KC], F16)
            ph = cst.tile([128, MT * 128], F16)
            # DMAs ordered by when each tensor gates the pipeline; only the
            # non-compute queues (SP sync + Pool gpsimd) issue DMAs — issuing
            # from the Scalar queue delays the casts behind DGE setup.
            nc.sync.dma_start(st[:], d_st[:])
            nc.sync.dma_start(xt0[:], d_xt0[:])
            nc.gpsimd.dma_start(yt0[:, 1, :, :], d_yt0b[:])
            nc.sync.dma_start(yt0[:, 0, :, :], d_yt0a[:])
            nc.gpsimd.dma_start(ph[:], d_ph[:])
            nc.sync.dma_start(xt1[:], d_xt1[:])
            nc.gpsimd.dma_start(yt1[:], d_yt1[:])

            yt = [yt0[:], yt1[:]]

            # PE warm-up: keeps the PE busy through the DMA lead-in so the
            # DVFS ramp is preserved until real work arrives (any PE idle
            # gap resets the clock to mid speed for many microseconds).
            # Also preload the Scalar COPY activation table.
            nc.scalar.copy(dm2[:], dm[:, 0:2])
            wu = zpp.tile([128, 2, KC], F32, tag="z", name="wu")
            for _ in range(N_WARM_BIG):
                nc.tensor.matmul(wu[:, 0, 0:256], dm[:, 0:128], dm[:],
                                 start=True, stop=True)
            for _ in range(N_WARM_SMALL):
                nc.tensor.matmul(wu[:, 0, 0:64], dm[:, 0:128], dm[:, 0:64],
                                 start=True, stop=True)

            slots = [(kc, j) for kc in range(NKC) for j in range(MT)]
            state = {}

            def emit_front(s):
                kc, j = slots[s]
                wgt = st[:, j * 128:(j + 1) * 128]
                xtc = xt0 if kc == 0 else xt1
                pq = pqp.tile([128, 2, KC], F32, tag="pq")
                nc.tensor.matmul(pq[:, 0, :], wgt, xtc[:, 0, :],
                                 start=True, stop=True)
                nc.tensor.matmul(pq[:, 1, :], wgt, xtc[:, 1, :],
                                 start=True, stop=True)
                ct = ctp.tile([128, 2, KC], F16, tag="ct")
                nc.scalar.copy(ct[:], pq[:])
                # products: pr[s1,s2,k] = ct[s2]*yt4[s1,s2] with
                # yt4 = [[cy,-sy],[-sy,-cy]] -> [A, B'', D'', C''].
                # Two dual ops (im row first so the im-pair sel matmuls and
                # the end-of-chunk z_im copy start earlier); ct is already
                # the (pc,qc) pair so no broadcast is needed.
                # GpSimd is deliberately NOT used here: concurrent Pool+DVE
                # tensor ops thrash SBUF ports (~2.4x slowdown on both).
                pr = prp.tile([128, 2, 2, KC], F16, tag="pr")
                nc.vector.tensor_tensor(pr[:, 1, :, :], ct[:],
                                        yt[kc][:, 1, :, :], OP.mult)
                nc.vector.tensor_tensor(pr[:, 0, :, :], ct[:],
                                        yt[kc][:, 0, :, :], OP.mult)
                state[(kc, j)] = pr

            def emit_sel(s):
                kc, j = slots[s]
                pr = state.pop((kc, j))
                if j == 0:
                    state[("z", kc)] = zpp.tile([128, 2, KC], F32, tag="z",
                                                name=f"z{kc}")
                z = state[("z", kc)]
                phj = ph[:, j * 128:(j + 1) * 128]
                # pr cols: [0,0]=A, [0,1]=B'' -> z_re; [1,0]=D'', [1,1]=C'' -> z_im
                # im-pair first so the z_im copy can start in parallel with
                # the z_re copy at the end of each chunk.
                nc.tensor.matmul(z[:, 1, :], phj, pr[:, 1, 0, :],
                                 start=(j == 0), stop=False,
                                 skip_group_check=True)
                nc.tensor.matmul(z[:, 1, :], phj, pr[:, 1, 1, :],
                                 start=False, stop=(j == MT - 1),
                                 skip_group_check=True)
                nc.tensor.matmul(z[:, 0, :], phj, pr[:, 0, 0, :],
                                 start=(j == 0), stop=False,
                                 skip_group_check=True)
                nc.tensor.matmul(z[:, 0, :], phj, pr[:, 0, 1, :],
                                 start=False, stop=(j == MT - 1),
                                 skip_group_check=True)

            def emit_zout(kc):
                z = state.pop(("z", kc))
                ks = slice(kc * KC, (kc + 1) * KC)
                zs = zsb.tile([128, 2, KC], F16, tag="zs")
                # im-half finishes first (im-pair sel runs first): copy and
                # DMA it immediately so the final DMA isn't gated on both
                nc.vector.tensor_copy(zs[:, 1, :], z[:, 1, :])
                nc.sync.dma_start(d_z[:, 1, ks], zs[:, 1, :])
                nc.scalar.copy(zs[:, 0, :], z[:, 0, :])
                nc.sync.dma_start(d_z[:, 0, ks], zs[:, 0, :])

            LAG = 4
            for s in range(len(slots)):
                if s == 2:
                    # PE bubble filler: stage-1 of slot 2 waits for cast-0 to
                    # free its PSUM tile; idle here resets the DVFS ramp.
                    for _ in range(N_FILL):
                        nc.tensor.matmul(wu[:, 0, 0:64], dm[:, 0:128],
                                         dm[:, 0:64], start=True, stop=True)
                emit_front(s)
                if s >= LAG:
                    emit_sel(s - LAG)
                    if slots[s - LAG][1] == MT - 1:
                        emit_zout(slots[s - LAG][0])
            for s in range(len(slots) - LAG, len(slots)):
                emit_sel(s)
                if slots[s][1] == MT - 1:
                    emit_zout(slots[s][0])

    nc.finalize()
    return nc


def _get_nc():
    if "nc" not in _CACHE:
        _CACHE["nc"] = _build_nc()
    return _CACHE["nc"]


def _stage_inputs(x, trj, phi, mps, sqrt_dcf):
    f16 = np.float16
    gy = (np.arange(H) - H // 2).astype(np.float64)
    gx = (np.arange(W) - W // 2).astype(np.float64)

    # sT[w, (a,c,h)] = x[a,h,w]*mps[c,h,w]
    s4 = (x[:, None, :, :] * mps[None, :, :, :]).astype(np.float64)  # [a,c,h,w]
    sT = np.ascontiguousarray(s4.transpose(3, 0, 1, 2).reshape(W, ACH)).astype(f16)

    # PH[p, j*128 + t*4 + c'] = phi[a,t] iff c'==c, with ac=2j+p//64
    PH = np.zeros((128, MT * 128), f16)
    phif = phi.astype(np.float64)
    for j in range(MT):
        for half in range(2):
            ac = 2 * j + half
            a, c = divmod(ac, C)
            rows = slice(half * 64, (half + 1) * 64)
            cols = j * 128 + np.arange(T) * C + c
            PH[rows, cols[None, :].repeat(64, 0)] = phif[a][None, :].astype(f16)

    in_maps = []
    for r in range(N_CORES):
        ty = trj[r, 0, :].astype(np.float64)
        tx = trj[r, 1, :].astype(np.float64)
        dcf = sqrt_dcf[r].astype(np.float64)
        py = ty[None, :] * gy[:, None]          # [64, K]
        px = tx[None, :] * gx[:, None]
        cy, sy = np.cos(py), np.sin(py)
        cxd, sxd = np.cos(px) * dcf, np.sin(px) * dcf
        cy2 = np.concatenate([cy, cy], 0)       # [128, K]
        sy2 = np.concatenate([sy, sy], 0)
        # yt4[p, s1, s2, k] = [[cy, -sy], [-sy, -cy]]
        yt4 = np.stack([cy2, -sy2, -sy2, -cy2], 1).reshape(128, 2, 2, K)
        yt4 = yt4.astype(f16)
        xt2 = np.stack([cxd, sxd], 1).astype(f16)          # [64, 2, K]
        m = {"st": sT, "ph": PH,
             "xt0": np.ascontiguousarray(xt2[:, :, :KC]),
             "xt1": np.ascontiguousarray(xt2[:, :, KC:]),
             "yt0a": np.ascontiguousarray(yt4[:, 0, :, :KC]),
             "yt0b": np.ascontiguousarray(yt4[:, 1, :, :KC]),
             "yt1": np.ascontiguousarray(yt4[:, :, :, KC:])}
        in_maps.append(m)
    return in_maps


def kernel(x, trj, phi, mps, sqrt_dcf, subsamp_idx, _trace=False):
    from concourse.bass_utils import run_bass_kernel_spmd

    nc = _get_nc()
    in_maps = _stage_inputs(np.asarray(x), np.asarray(trj), np.asarray(phi),
                            np.asarray(mps), np.asarray(sqrt_dcf))
    res = run_bass_kernel_spmd(nc, in_maps, core_ids=list(range(N_CORES)),
                               trace=_trace)
    out = np.empty((T, C, K), dtype=np.complex64)
    idx = np.asarray(subsamp_idx).astype(np.int64)
    for t in range(T):
        z = res.results[int(idx[t])]["z"].astype(np.float32)
        for c in range(C):
            out[t, c, :] = z[t * 4 + c, 0] + 1j * z[t * 4 + c, 1]
    if _trace:
        kernel._last_results = res
    return out
